# revision 4
# baseline (speedup 1.0000x reference)
"""Cayley orthogonal transform kernel for Trainium2 (8 NeuronCores).

Math: per head h, y = (I - S) ((1+eps) I + S)^{-1} x applied along D=128,
where S = S_raw - S_raw^T is skew-symmetric.

Strategy:
  * Host: skew-symmetrize S_raw, and lay x out as xT[h, d, token] (token-major
    per head) so the device only ever runs plain matmuls - no on-device
    transposes.  Heads are sharded 2-per-core across the 8 cores (tensor
    parallel, embarrassingly parallel per the problem structure).
  * Device (per core): build W^T = ((1+eps)I - S)^{-1} (I + S) per head with a
    Newton-Schulz iteration (pure 128x128 matmuls, converges to fp32 accuracy
    in <=7 iterations since ||S||_2 ~ 1.6), then stream the (128 x 16384)
    token panel through the PE array in 512-column fp32 matmuls:
        yT[h] = W @ xT[h]
    PSUM results are evacuated to SBUF alternating Vector/Scalar engines and
    DMA'd back to DRAM in 2 MiB chunks.  The kernel is HBM-bandwidth bound
    (~34 MB of DRAM traffic per core); all compute hides under the DMA.
  * Host: inverse layout transform back to (B, H, N, D).
"""

import os
import sys

import numpy as np

B, H, N, D = 4, 16, 4096, 128
N_CORES = 8
HPC = H // N_CORES          # heads per core
T = B * N                   # tokens per head
CHUNK = 4096                # columns per DMA tile (2 MiB)
MM = 512                    # columns per fp32 matmul (one PSUM bank)
NS_ITERS = 7                # Newton-Schulz iterations
NS_C = 0.42                 # NS initial scale: X0 = c * G^T  (safe for ||S||<~1.9)
EPS = 1e-5

_CACHE = {}


def _ensure_path():
    for p in ("/opt/trn_rl_repo", "/root/.axon_site/_ro/trn_rl_repo"):
        if os.path.isdir(p) and p not in sys.path:
            sys.path.insert(0, p)
    _install_ntff_hook()


def _install_ntff_hook():
    """The agent image's ``antenv`` lacks ``axon_hooks``, which makes
    ``run_bass_kernel_spmd(trace=True)`` crash instead of degrading.  Provide
    the module and register the ctypes NTFF hook the boot shim would have."""
    if "antenv.axon_hooks" in sys.modules:
        return
    try:
        import types

        import antenv

        if hasattr(antenv, "axon_hooks"):
            return
        mod = types.ModuleType("antenv.axon_hooks")
        state = {"hook": None}
        mod.set_axon_ntff_profile_hook = lambda h: state.__setitem__("hook", h)
        mod.get_axon_ntff_profile_hook = lambda: state["hook"]
        sys.modules["antenv.axon_hooks"] = mod
        antenv.axon_hooks = mod
        try:
            from trn_agent_boot.trn_boot import _ntff_profile_via_ctypes

            so_path = "/opt/axon/libaxon_pjrt.so"
            if os.path.exists(so_path):
                mod.set_axon_ntff_profile_hook(_ntff_profile_via_ctypes(so_path))
        except Exception:
            pass  # hook stays None -> concourse logs + skips tracing
    except Exception:
        pass


def _build_nc():
    """Build the (single-program SPMD) Bass kernel for one core's shard."""
    _ensure_path()
    import concourse.tile as tile
    from concourse import bacc, mybir
    from concourse.masks import make_identity

    f32 = mybir.dt.float32
    Alu = mybir.AluOpType

    nc = bacc.Bacc("TRN2", target_bir_lowering=False, debug=False)
    xT_d = nc.dram_tensor("xT", [HPC * D, T], f32, kind="ExternalInput").ap()
    s_d = nc.dram_tensor("s", [HPC * D, D], f32, kind="ExternalInput").ap()
    yT_d = nc.dram_tensor("yT", [HPC * D, T], f32, kind="ExternalOutput").ap()

    with tile.TileContext(nc) as tc:
        with (
            tc.tile_pool(name="const", bufs=1) as const_pool,
            tc.tile_pool(name="ns", bufs=2) as ns_pool,
            tc.tile_pool(name="wt", bufs=1) as wt_pool,
            tc.tile_pool(name="xin", bufs=6) as in_pool,
            tc.tile_pool(name="yout", bufs=3) as out_pool,
            tc.tile_pool(name="mmps", bufs=4, space="PSUM") as ps_big,
            tc.tile_pool(name="nsps", bufs=4, space="PSUM") as ps_ns,
        ):
            ident = const_pool.tile([D, D], f32, tag="ident")
            make_identity(nc, ident)
            twoE = const_pool.tile([D, D], f32, tag="twoE")
            nc.vector.tensor_scalar_mul(twoE, ident, 2.0)

            # --- Newton-Schulz per head: WT = Ginv @ (I + S), G = (1+eps)I - S
            # bass matmul computes lhsT.T @ rhs; note A := (1+eps)I + S = G^T.
            wts = []
            for h in range(HPC):
                s_sb = const_pool.tile([D, D], f32, tag=f"s{h}")
                nc.sync.dma_start(out=s_sb, in_=s_d[h * D:(h + 1) * D, :])
                a_mat = const_pool.tile([D, D], f32, tag=f"amat{h}")
                nc.vector.scalar_tensor_tensor(
                    out=a_mat, in0=ident, scalar=1.0 + EPS, in1=s_sb,
                    op0=Alu.mult, op1=Alu.add)
                ips = const_pool.tile([D, D], f32, tag=f"ips{h}")
                nc.vector.tensor_add(ips, ident, s_sb)
                g_mat = const_pool.tile([D, D], f32, tag=f"g{h}")
                nc.vector.scalar_tensor_tensor(
                    out=g_mat, in0=ident, scalar=1.0 + EPS, in1=s_sb,
                    op0=Alu.mult, op1=Alu.subtract)

                X = ns_pool.tile([D, D], f32, tag=f"x{h}")
                nc.vector.tensor_scalar_mul(X, a_mat, NS_C)    # X0 = c G^T
                XT = ns_pool.tile([D, D], f32, tag=f"xt{h}")
                nc.vector.tensor_scalar_mul(XT, g_mat, NS_C)   # X0^T = c G

                for k in range(NS_ITERS):
                    t_ps = ps_ns.tile([D, D], f32, tag="nsps")
                    nc.tensor.matmul(t_ps, lhsT=a_mat, rhs=X, start=True, stop=True)  # G X
                    t2 = ns_pool.tile([D, D], f32, tag=f"t2{h}")
                    nc.vector.tensor_sub(t2, twoE, t_ps)       # 2I - G X
                    if k < NS_ITERS - 1:
                        xn_ps = ps_ns.tile([D, D], f32, tag="nsps")
                        nc.tensor.matmul(xn_ps, lhsT=XT, rhs=t2, start=True, stop=True)  # X T2
                        Xn = ns_pool.tile([D, D], f32, tag=f"x{h}")
                        nc.scalar.copy(Xn, xn_ps)
                        X = Xn
                    xtn_ps = ps_ns.tile([D, D], f32, tag="nsps")
                    nc.tensor.matmul(xtn_ps, lhsT=t2, rhs=XT, start=True, stop=True)  # (X T2)^T
                    XTn = ns_pool.tile([D, D], f32, tag=f"xt{h}")
                    nc.scalar.copy(XTn, xtn_ps)
                    XT = XTn

                wt_ps = ps_ns.tile([D, D], f32, tag="nsps")
                nc.tensor.matmul(wt_ps, lhsT=XT, rhs=ips, start=True, stop=True)  # Ginv (I+S)
                wt = wt_pool.tile([D, D], f32, tag=f"wt{h}")
                nc.vector.tensor_copy(wt, wt_ps)
                wts.append(wt)

            # --- streaming panel matmul: yT[h] = W @ xT[h]
            for h in range(HPC):
                r0 = h * D
                for ci in range(T // CHUNK):
                    c0 = ci * CHUNK
                    xin = in_pool.tile([D, CHUNK], f32, tag="xin")
                    nc.sync.dma_start(out=xin, in_=xT_d[r0:r0 + D, c0:c0 + CHUNK])
                    yout = out_pool.tile([D, CHUNK], f32, tag="yout")
                    for j in range(CHUNK // MM):
                        ps = ps_big.tile([D, MM], f32, tag="mm")
                        nc.tensor.matmul(ps, lhsT=wts[h], rhs=xin[:, j * MM:(j + 1) * MM],
                                         start=True, stop=True)
                        if j % 2 == 0:
                            nc.vector.tensor_copy(yout[:, j * MM:(j + 1) * MM], ps)
                        else:
                            nc.scalar.copy(yout[:, j * MM:(j + 1) * MM], ps)
                    nc.scalar.dma_start(out=yT_d[r0:r0 + D, c0:c0 + CHUNK], in_=yout)
    nc.compile()
    return nc


def _get_nc():
    if "nc" not in _CACHE:
        _CACHE["nc"] = _build_nc()
    return _CACHE["nc"]


def _prep_inputs(x, S_raw):
    """Host-side shard + layout prep. Returns per-core input maps."""
    x = np.asarray(x, dtype=np.float32)
    S_raw = np.asarray(S_raw, dtype=np.float32)
    S = S_raw - S_raw.transpose(0, 2, 1)
    # (B,H,N,D) -> (H, D, B*N), token-major per head
    xT_full = np.ascontiguousarray(x.transpose(1, 3, 0, 2)).reshape(H * D, T)
    S_full = np.ascontiguousarray(S).reshape(H * D, D)
    in_maps = []
    for c in range(N_CORES):
        r = c * HPC * D
        in_maps.append({
            "xT": xT_full[r:r + HPC * D],
            "s": S_full[r:r + HPC * D],
        })
    return in_maps


def _postprocess(results):
    """Gather per-core yT shards back into (B, H, N, D)."""
    yT_full = np.concatenate([r["yT"] for r in results], axis=0)  # (H*D, T)
    y = yT_full.reshape(H, D, B, N).transpose(2, 0, 3, 1)
    return np.ascontiguousarray(y)


def _execute(in_maps, trace=False, **kwargs):
    _ensure_path()
    from concourse.bass_utils import run_bass_kernel_spmd

    nc = _get_nc()
    return run_bass_kernel_spmd(nc, in_maps, core_ids=list(range(N_CORES)),
                                trace=trace, **kwargs)


def kernel(x, S_raw):
    in_maps = _prep_inputs(x, S_raw)
    res = _execute(in_maps)
    return _postprocess(res.results)


# revision 8
# speedup vs baseline: 1.1495x; 1.1495x over previous
"""Cayley orthogonal transform kernel for Trainium2 (8 NeuronCores).

Math: per head h, y = (I - S) ((1+eps) I + S)^{-1} x applied along D=128,
where S = S_raw - S_raw^T is skew-symmetric.

Strategy:
  * Host: skew-symmetrize S_raw, and lay x out as xT[h, d, token] (token-major
    per head) so the device only ever runs plain matmuls - no on-device
    transposes.  Heads are sharded 2-per-core across the 8 cores (tensor
    parallel, embarrassingly parallel per the problem structure).
  * Device (per core): build W^T = ((1+eps)I - S)^{-1} (I + S) per head with a
    Newton-Schulz iteration (pure 128x128 matmuls, converges to fp32 accuracy
    in <=7 iterations since ||S||_2 ~ 1.6), then stream the (128 x 16384)
    token panel through the PE array in 512-column fp32 matmuls:
        yT[h] = W @ xT[h]
    PSUM results are evacuated to SBUF alternating Vector/Scalar engines and
    DMA'd back to DRAM in 2 MiB chunks.  The kernel is HBM-bandwidth bound
    (~34 MB of DRAM traffic per core); all compute hides under the DMA.
  * Host: inverse layout transform back to (B, H, N, D).
"""

import os
import sys

import numpy as np

B, H, N, D = 4, 16, 4096, 128
N_CORES = 8
HPC = H // N_CORES          # heads per core
T = B * N                   # tokens per head
CHUNK = 4096                # columns per DMA tile (2 MiB)
MM = 512                    # columns per fp32 matmul (one PSUM bank)
NS_ITERS = 7                # Newton-Schulz iterations
NS_C = 0.42                 # NS initial scale: X0 = c * G^T  (safe for ||S||<~1.9)
EPS = 1e-5

_CACHE = {}


def _ensure_path():
    for p in ("/opt/trn_rl_repo", "/root/.axon_site/_ro/trn_rl_repo"):
        if os.path.isdir(p) and p not in sys.path:
            sys.path.insert(0, p)
    _install_ntff_hook()


def _install_ntff_hook():
    """The agent image's ``antenv`` lacks ``axon_hooks``, which makes
    ``run_bass_kernel_spmd(trace=True)`` crash instead of degrading.  Provide
    the module and register the ctypes NTFF hook the boot shim would have."""
    if "antenv.axon_hooks" in sys.modules:
        return
    try:
        import types

        import antenv

        if hasattr(antenv, "axon_hooks"):
            return
        mod = types.ModuleType("antenv.axon_hooks")
        state = {"hook": None}
        mod.set_axon_ntff_profile_hook = lambda h: state.__setitem__("hook", h)
        mod.get_axon_ntff_profile_hook = lambda: state["hook"]
        sys.modules["antenv.axon_hooks"] = mod
        antenv.axon_hooks = mod
        try:
            from trn_agent_boot.trn_boot import _ntff_profile_via_ctypes

            so_path = "/opt/axon/libaxon_pjrt.so"
            if os.path.exists(so_path):
                mod.set_axon_ntff_profile_hook(_ntff_profile_via_ctypes(so_path))
        except Exception:
            pass  # hook stays None -> concourse logs + skips tracing
    except Exception:
        pass


def _build_nc():
    """Build the (single-program SPMD) Bass kernel for one core's shard."""
    _ensure_path()
    import concourse.tile as tile
    from concourse import bacc, mybir
    from concourse.masks import make_identity

    f32 = mybir.dt.float32
    f32r = mybir.dt.float32r
    Alu = mybir.AluOpType

    nc = bacc.Bacc("TRN2", target_bir_lowering=False, debug=False)
    xT_d = nc.dram_tensor("xT", [HPC * D, T], f32r, kind="ExternalInput").ap()
    s_d = nc.dram_tensor("s", [HPC * D, D], f32, kind="ExternalInput").ap()
    yT_d = nc.dram_tensor("yT", [HPC * D, T], f32, kind="ExternalOutput").ap()

    with tile.TileContext(nc) as tc:
        with (
            tc.tile_pool(name="const", bufs=1) as const_pool,
            tc.tile_pool(name="ns", bufs=2) as ns_pool,
            tc.tile_pool(name="wt", bufs=1) as wt_pool,
            tc.tile_pool(name="xin", bufs=6) as in_pool,
            tc.tile_pool(name="yout", bufs=3) as out_pool,
            tc.tile_pool(name="mmps", bufs=4, space="PSUM") as ps_big,
            tc.tile_pool(name="nsps", bufs=4, space="PSUM") as ps_ns,
        ):
            ident = const_pool.tile([D, D], f32, tag="ident")
            make_identity(nc, ident)
            twoE = const_pool.tile([D, D], f32, tag="twoE")
            nc.vector.tensor_scalar_mul(twoE, ident, 2.0)

            # --- Newton-Schulz per head: WT = Ginv @ (I + S), G = (1+eps)I - S
            # bass matmul computes lhsT.T @ rhs; note A := (1+eps)I + S = G^T.
            wts = []
            for h in range(HPC):
                s_sb = const_pool.tile([D, D], f32, tag=f"s{h}")
                nc.sync.dma_start(out=s_sb, in_=s_d[h * D:(h + 1) * D, :])
                a_mat = const_pool.tile([D, D], f32, tag=f"amat{h}")
                nc.vector.scalar_tensor_tensor(
                    out=a_mat, in0=ident, scalar=1.0 + EPS, in1=s_sb,
                    op0=Alu.mult, op1=Alu.add)
                ips = const_pool.tile([D, D], f32, tag=f"ips{h}")
                nc.vector.tensor_add(ips, ident, s_sb)
                g_mat = const_pool.tile([D, D], f32, tag=f"g{h}")
                nc.vector.scalar_tensor_tensor(
                    out=g_mat, in0=ident, scalar=1.0 + EPS, in1=s_sb,
                    op0=Alu.mult, op1=Alu.subtract)

                X = ns_pool.tile([D, D], f32, tag=f"x{h}")
                nc.vector.tensor_scalar_mul(X, a_mat, NS_C)    # X0 = c G^T
                XT = ns_pool.tile([D, D], f32, tag=f"xt{h}")
                nc.vector.tensor_scalar_mul(XT, g_mat, NS_C)   # X0^T = c G

                for k in range(NS_ITERS):
                    t_ps = ps_ns.tile([D, D], f32, tag="nsps")
                    nc.tensor.matmul(t_ps, lhsT=a_mat, rhs=X, start=True, stop=True)  # G X
                    t2 = ns_pool.tile([D, D], f32, tag=f"t2{h}")
                    nc.vector.tensor_sub(t2, twoE, t_ps)       # 2I - G X
                    if k < NS_ITERS - 1:
                        xn_ps = ps_ns.tile([D, D], f32, tag="nsps")
                        nc.tensor.matmul(xn_ps, lhsT=XT, rhs=t2, start=True, stop=True)  # X T2
                        Xn = ns_pool.tile([D, D], f32, tag=f"x{h}")
                        nc.scalar.copy(Xn, xn_ps)
                        X = Xn
                    xtn_ps = ps_ns.tile([D, D], f32, tag="nsps")
                    nc.tensor.matmul(xtn_ps, lhsT=t2, rhs=XT, start=True, stop=True)  # (X T2)^T
                    XTn = ns_pool.tile([D, D], f32, tag=f"xt{h}")
                    nc.scalar.copy(XTn, xtn_ps)
                    XT = XTn

                wt_ps = ps_ns.tile([D, D], f32, tag="nsps")
                nc.tensor.matmul(wt_ps, lhsT=XT, rhs=ips, start=True, stop=True)  # Ginv (I+S)
                wt = wt_pool.tile([D, D], f32r, tag=f"wt{h}")
                nc.vector.tensor_copy(wt, wt_ps)
                wts.append(wt)

            # --- streaming panel matmul: yT[h] = W @ xT[h]
            # float32r runs the PE at 1 cycle/row for free-dim >= 256 (vs 4
            # for plain fp32), which keeps the whole kernel DMA-bound.
            half = CHUNK // 2
            for h in range(HPC):
                r0 = h * D
                for ci in range(T // CHUNK):
                    c0 = ci * CHUNK
                    xin = in_pool.tile([D, CHUNK], f32r, tag="xin")
                    nc.sync.dma_start(out=xin, in_=xT_d[r0:r0 + D, c0:c0 + CHUNK])
                    yout = out_pool.tile([D, CHUNK], f32, tag="yout")
                    for j in range(CHUNK // MM):
                        ps = ps_big.tile([D, MM], f32, tag="mm")
                        nc.tensor.matmul(ps, lhsT=wts[h],
                                         rhs=xin[:, j * MM:(j + 1) * MM],
                                         start=True, stop=True)
                        if j % 2 == 0:
                            nc.vector.tensor_copy(yout[:, j * MM:(j + 1) * MM], ps)
                        else:
                            nc.scalar.copy(yout[:, j * MM:(j + 1) * MM], ps)
                    # two half-stores so the DMA overlaps the second half's evac
                    nc.scalar.dma_start(out=yT_d[r0:r0 + D, c0:c0 + half],
                                        in_=yout[:, 0:half])
                    nc.scalar.dma_start(out=yT_d[r0:r0 + D, c0 + half:c0 + CHUNK],
                                        in_=yout[:, half:CHUNK])
    nc.compile()
    return nc


def _get_nc():
    if "nc" not in _CACHE:
        _CACHE["nc"] = _build_nc()
    return _CACHE["nc"]


def _prep_inputs(x, S_raw):
    """Host-side shard + layout prep. Returns per-core input maps."""
    x = np.asarray(x, dtype=np.float32)
    S_raw = np.asarray(S_raw, dtype=np.float32)
    S = S_raw - S_raw.transpose(0, 2, 1)
    # (B,H,N,D) -> (H, D, B*N), token-major per head
    xT_full = np.ascontiguousarray(x.transpose(1, 3, 0, 2)).reshape(H * D, T)
    S_full = np.ascontiguousarray(S).reshape(H * D, D)
    in_maps = []
    for c in range(N_CORES):
        r = c * HPC * D
        in_maps.append({
            "xT": xT_full[r:r + HPC * D],
            "s": S_full[r:r + HPC * D],
        })
    return in_maps


def _postprocess(results):
    """Gather per-core yT shards back into (B, H, N, D)."""
    yT_full = np.concatenate([r["yT"] for r in results], axis=0)  # (H*D, T)
    y = yT_full.reshape(H, D, B, N).transpose(2, 0, 3, 1)
    return np.ascontiguousarray(y)


def _execute(in_maps, trace=False, **kwargs):
    _ensure_path()
    from concourse.bass_utils import run_bass_kernel_spmd

    nc = _get_nc()
    return run_bass_kernel_spmd(nc, in_maps, core_ids=list(range(N_CORES)),
                                trace=trace, **kwargs)


def kernel(x, S_raw):
    in_maps = _prep_inputs(x, S_raw)
    res = _execute(in_maps)
    return _postprocess(res.results)
